# revision 44
# baseline (speedup 1.0000x reference)
"""Trainium2 Bass kernel for nn_CoCa4Traj (parallel transformer block + cross attention).

Sharding: 8 cores = 4 batches x 2 interleaved sequence halves.  Core (b, s)
owns query blocks {s, s+2, s+4, s+6} (128 tokens each) of batch b; causal K/V
are recomputed for the full sequence on every core (MQA k/v is a single head,
so this is cheap) -- no collectives, one uniform SPMD program.

v2 design notes (all aimed at a continuously-busy PE at its max p-state --
the TRN2 tensor engine only reaches 2.4 GHz after ~3us of gapless work):
  * the FF phase is emitted right after the hc layernorm folds; the rest of
    the S0 tail (kv/kv2 folds, v transposes, q folds) is spread BETWEEN FF
    weight chunks so its PE bits ride the FF stream; FF weights prefetch
    from the first instruction
  * x-stats run in 512-token chunks: chunk 0 is exactly the own tokens, so
    the hc folds (and with them the FF) unblock after one chunk
  * causal mask is applied as a 0/1 fp16 MULTIPLY on the post-exp probs
    (first key-block only); no fp32 PSUM adds, no -1e30 logits
  * MQA AV uses one augmented-V stationary (v|ones) for BOTH heads: two
    65-row matmuls put each head's AV and its softmax denominator in the
    same psum tile, denominators together on partition 64; AV is deferred
    THREE key-blocks behind the sims and the 32-matmul ff_out blob is
    spread through the loop so exp+mask latency never stalls the PE
  * normalization: one DVE reciprocal_approx_fast over both denominator rows,
    two gpsimd partition_broadcasts, two DVE multiplies; head-1's 64 rows are
    moved into place with an SBUF->SBUF partition-shift DMA (no DRAM bounce,
    no 6-cycle-per-element exact reciprocal)
  * per-token layernorm statistics: sums via ones-matmuls, squares on the
    DVE (fp16 2x mode); every LN application is folded into the consuming
    projection as a per-token rescale + rank-1 correction; rsqrt is a
    single-table AF.Sqrt + DVE approx reciprocal (table loads only at the
    five phase boundaries)
  * S3 runs as three passes (attn_out, y stats, raw q2 projection) with a
    4-deep psum ring so the PE never waits on the trailing DVE fold chain
  * PSUM banks are budgeted per phase (S0 4 + FF 4; sim 4 + av 2 + blob 2)
    so no phase ever waits on a bank whose last reader is a late fold
"""

import sys

sys.path.insert(0, "/opt/trn_rl_repo")

import numpy as np

import concourse.bass as bass
import concourse.mybir as mybir
import concourse.tile as tile
from concourse import bacc
from concourse.bass_utils import run_bass_kernel_spmd
from concourse.masks import make_identity

F32 = mybir.dt.float32
F16 = mybir.dt.float16
AF = mybir.ActivationFunctionType
ALU = mybir.AluOpType

P = 128
D = 1024
ND = 8
H = 16
DH = 64
TOWN = 512          # tokens owned per core
TALL = 1024         # full sequence
TOT = TALL + TOWN   # own-first concatenated token axis: [own 512 | all 1024]
CTX = 512
NT = TOT // P       # 12 token-major x tiles
NCT = CTX // P      # 4 token-major ctx tiles
NJB = 8             # self-attn key blocks
NJ2 = 4             # cross-attn key blocks
NG = 32             # ff groups of 128
SCALE = DH ** -0.5
EPS = 1e-5
NCORES = 8

LAST_EXEC_NS = None
TRACE = False
_CACHE = {}


def _row_stats(nc, pool, srow, qrow, n, eps_col, pref):
    """srow/qrow [1, n] SBUF: per-token sums / sums of squares over the D
    features.  Returns (arow, brow) [1, n] fp16 rows on partition 0 with
    a = rsqrt(var+eps) via single-table AF.Sqrt + DVE approx reciprocal,
    b = -mean*a."""
    mur = pool.tile([1, n], F16, tag=pref + "mu", name=pref + "mu")
    nc.vector.tensor_scalar_mul(mur[:], srow[:], 1.0 / D)
    m2r = pool.tile([1, n], F32, tag=pref + "m2", name=pref + "m2")
    nc.vector.tensor_mul(m2r[:], mur[:], mur[:])
    nc.vector.scalar_tensor_tensor(m2r[:], qrow[:], 1.0 / D, m2r[:],
                                   op0=ALU.mult, op1=ALU.subtract)
    nc.scalar.activation(m2r[:], m2r[:], AF.Sqrt, bias=eps_col[0:1, :])
    ar32 = pool.tile([1, n], F32, tag=pref + "a3", name=pref + "a3")
    nc.vector.reciprocal_approx_fast(ar32[:], m2r[:])
    arow = pool.tile([1, n], F16, tag=pref + "ar", name=pref + "ar")
    nc.scalar.copy(arow[:], ar32[:])
    brow = pool.tile([1, n], F16, tag=pref + "br", name=pref + "br")
    nc.vector.scalar_tensor_tensor(brow[:], mur[:], -1.0, ar32[:],
                                   op0=ALU.mult, op1=ALU.mult)
    return arow, brow


def build_module():
    nc = bacc.Bacc("TRN2", target_bir_lowering=False, debug=False)

    def din(name, shape, dt=F16):
        return nc.dram_tensor(name, shape, dt, kind="ExternalInput").ap()

    xc_d = din("xc", [D, TOT])
    xo_d = din("xo", [D, TOWN], F32)
    ctf_d = din("ctf", [D, CTX])
    wq1_d = din("wq1", [ND, P, ND, P])
    wkv1_d = din("wkv1", [P, ND, P])
    wff_d = din("wff", [8, P, ND, 1024])
    wattn_d = din("wattn", [ND, P, ND, P])
    wffo_d = din("wffo", [ND, P, NG, P])
    wq2_d = din("wq2", [ND, P, ND, P])
    wkv2_d = din("wkv2", [P, ND, P])
    wout_d = din("wout", [ND, P, ND, P])
    mask_d = din("mask", [P, NJB, P], F16)
    cq1_d = din("cq1", [P, ND], F32)
    ckv1_d = din("ckv1", [P, 1], F32)
    c2_d = din("c2", [P, ND], F32)
    ckv2_d = din("ckv2", [P, 1], F32)
    out_d = nc.dram_tensor("outT", [D, TOWN], F32, kind="ExternalOutput").ap()

    def w4(dram, i):
        return dram[i:i + 1].rearrange("o p d f -> (o p) d f")

    def attn_norm(nc, pool, avp, out_tile, pref):
        """avp [65, 2, TOWN] psum: head h AV at rows 0:64 of [:, h, :], its
        softmax denominator on row 64.  One approx reciprocal over both
        denominator rows, two gpsimd broadcasts, two DVE multiplies; head 1
        is shifted into rows 64:128 of out_tile with an SBUF->SBUF DMA."""
        den = pool.tile([1, 2, TOWN], F32, tag=pref + "den", name=pref + "den")
        nc.vector.tensor_copy(den[0:1, :, :].rearrange("p a b -> p (a b)"),
                              avp[DH:DH + 1, :, :].rearrange("p a b -> p (a b)"))
        rcp = pool.tile([1, 2, TOWN], F32, tag=pref + "rcp", name=pref + "rcp")
        nc.vector.reciprocal_approx_fast(
            rcp[0:1, :, :].rearrange("p a b -> p (a b)"),
            den[0:1, :, :].rearrange("p a b -> p (a b)"))
        rb = pool.tile([DH, 2, TOWN], F32, tag=pref + "rb", name=pref + "rb")
        nc.gpsimd.partition_broadcast(rb[:, 0, :], rcp[0:1, 0, :])
        nc.gpsimd.partition_broadcast(rb[:, 1, :], rcp[0:1, 1, :])
        nc.vector.tensor_mul(out_tile[0:DH, :], avp[0:DH, 0, :], rb[:, 0, :])
        stag = pool.tile([DH, TOWN], F16, tag=pref + "st", name=pref + "st")
        nc.vector.tensor_mul(stag[:], avp[0:DH, 1, :], rb[:, 1, :])
        nc.sync.dma_start(out_tile[DH:P, :], stag[:])

    with tile.TileContext(nc, pool_alloc_mode="queue") as tc:
      with tc.tile_pool(name="consts", bufs=1) as consts, \
           tc.tile_pool(name="acts", bufs=1) as acts:
        ident = consts.tile([P, P], F16, tag="ident", name="ident")
        make_identity(nc, ident[:])
        eps_col = consts.tile([P, 1], F32, tag="eps", name="eps")
        nc.gpsimd.memset(eps_col[:], EPS)
        ones_h = consts.tile([P, 1], F16, tag="ones", name="ones")
        nc.gpsimd.memset(ones_h[:], 1.0)

        # ---- persistent activation tiles ----
        qT = [acts.tile([P, TOWN], F16, tag=f"qT{i}", name=f"qT{i}") for i in range(ND)]
        kb = acts.tile([P, TALL], F16, tag="kb", name="kb")
        v_aug = [acts.tile([P, DH + 1], F16, tag=f"va{i}", name=f"va{i}") for i in range(NJB)]
        q2T = [acts.tile([P, TOWN], F16, tag=f"q2{i}", name=f"q2{i}") for i in range(ND)]
        k2b = acts.tile([P, CTX], F16, tag="k2b", name="k2b")
        v2_aug = [acts.tile([P, DH + 1], F16, tag=f"v2a{i}", name=f"v2a{i}") for i in range(NJ2)]
        out2b = [acts.tile([P, TOWN], F16, tag=f"o2b{i}", name=f"o2b{i}") for i in range(ND)]
        hc = [acts.tile([P, TOWN], F16, tag=f"hc{i}", name=f"hc{i}") for i in range(ND)]
        a2b = acts.tile([P, TOWN], F16, tag="a2b", name="a2b")
        b2b = acts.tile([P, TOWN], F16, tag="b2b", name="b2b")

        wff_tiles = {}

        def load_wff(gc):
            if gc < 8:
                w = wffp.tile([P, ND, 1024], F16, tag="wff", name="wff")
                nc.sync.dma_start(w[:], w4(wff_d, gc))
                wff_tiles[gc] = w

        with tc.tile_pool(name="sgp", bufs=1) as sgp, \
             tc.tile_pool(name="wfop", bufs=2) as wfop:
          sgT = {}
          wffo_tiles = {}
          _wffp_cm = tc.tile_pool(name="wffp", bufs=2)
          wffp = _wffp_cm.__enter__()

          def load_wfo(fp):
              if fp < ND:
                  t = wfop.tile([P, NG, P], F16, tag="wfo", name="wfo")
                  nc.sync.dma_start(t[:], w4(wffo_d, fp))
                  wffo_tiles[fp] = t

          # ====== S0 + S2a: stats, folded projections, FF phase ============
          # S0 psum pools use 4 banks; the FF psum pools get the other 4, so
          # the FF matmul stream starts the moment hc is folded.  The S0
          # tail (kv/ctx/kv2 folds, transposes, q folds -- only needed by
          # S2b/S4) is emitted BETWEEN FF chunks so its PE bits ride the FF
          # stream and its vector/scalar bits fill spare cycles.
          with tc.tile_pool(name="s0", bufs=1) as s0, \
               tc.tile_pool(name="wq1p", bufs=2) as wq1p, \
               tc.tile_pool(name="stps", bufs=1, space="PSUM") as stps, \
               tc.tile_pool(name="vtps", bufs=1, space="PSUM") as vtps, \
               tc.tile_pool(name="s1ps", bufs=1, space="PSUM") as s1ps, \
               tc.tile_pool(name="silp", bufs=2) as silp, \
               tc.tile_pool(name="ffap", bufs=3, space="PSUM") as ffa_ps, \
               tc.tile_pool(name="ffgp", bufs=2, space="PSUM") as ffg_ps:
            _xcp_cm = tc.tile_pool(name="xcp", bufs=1)
            xcp = _xcp_cm.__enter__()
            xcs = [xcp.tile([P, TOT], F16, tag=f"xc{i}", name=f"xc{i}") for i in range(ND)]
            for i in range(ND):
                eng = nc.sync if i % 2 == 0 else nc.scalar
                eng.dma_start(xcs[i][:], xc_d[i * P:(i + 1) * P, :])
            mask_sb = consts.tile([P, NJB, P], F16, tag="mask", name="mask")
            nc.sync.dma_start(mask_sb[:], mask_d[:])
            cq1_sb = consts.tile([P, ND], F32, tag="cq1", name="cq1")
            nc.sync.dma_start(cq1_sb[:], cq1_d[:])
            ckv1_sb = consts.tile([P, 1], F32, tag="ckv1", name="ckv1")
            nc.sync.dma_start(ckv1_sb[:], ckv1_d[:])
            c2_sb = consts.tile([P, ND], F32, tag="c2", name="c2")
            nc.sync.dma_start(c2_sb[:], c2_d[:])
            ckv2_sb = consts.tile([P, 1], F32, tag="ckv2", name="ckv2")
            nc.sync.dma_start(ckv2_sb[:], ckv2_d[:])
            ctf = [s0.tile([P, CTX], F16, tag=f"ctf{i}", name=f"ctf{i}") for i in range(ND)]
            for i in range(ND):
                nc.sync.dma_start(ctf[i][:], ctf_d[i * P:(i + 1) * P, :])

            wq1_tiles = {}

            def load_wq1(fb):
                if fb < ND:
                    t = wq1p.tile([P, ND, P], F16, tag="wq1", name="wq1")
                    nc.sync.dma_start(t[:], w4(wq1_d, fb))
                    wq1_tiles[fb] = t

            load_wq1(0)
            load_wq1(1)
            load_wff(0)
            load_wff(1)

            # ---- x per-token stats: sums via ones-matmuls, squares on DVE.
            # Chunked by 512 tokens; chunk 0 is exactly the own tokens, so
            # the hc/q folds unblock after one chunk of row stats while the
            # kv fold (chunks 1-2) trails into the FF phase. ----
            srow = xcp.tile([1, TOT], F16, tag="srow", name="srow")
            qrow = xcp.tile([1, TOT], F16, tag="qrow", name="qrow")
            csrow = s0.tile([1, CTX], F16, tag="csrow", name="csrow")
            cqrow = s0.tile([1, CTX], F16, tag="cqrow", name="cqrow")
            a_b = s0.tile([P, TOT], F16, tag="a_b", name="a_b")
            b_b = s0.tile([P, TOT], F16, tag="b_b", name="b_b")

            def x_sum(c):
                cs = slice(c * 512, (c + 1) * 512)
                ps = stps.tile([1, 512], F32, tag="st", name="st")
                for d in range(ND):
                    nc.tensor.matmul(ps[:], ones_h[:], xcs[d][:, cs],
                                     start=(d == 0), stop=(d == ND - 1))
                nc.vector.tensor_copy(srow[0:1, cs], ps[:])

            def x_sqsum(c):
                cs = slice(c * 512, (c + 1) * 512)
                sqx = [xcp.tile([P, 512], F16, tag=f"sqx{d % 2}", name=f"sqx{d % 2}")
                       for d in range(ND)]
                for d in range(ND):
                    nc.vector.tensor_mul(sqx[d][:], xcs[d][:, cs], xcs[d][:, cs])
                ps = stps.tile([1, 512], F32, tag="st", name="st")
                for d in range(ND):
                    nc.tensor.matmul(ps[:], ones_h[:], sqx[d][:],
                                     start=(d == 0), stop=(d == ND - 1))
                nc.vector.tensor_copy(qrow[0:1, cs], ps[:])

            def x_rowstats(c):
                cs = slice(c * 512, (c + 1) * 512)
                arow, brow = _row_stats(nc, xcp, srow[0:1, cs], qrow[0:1, cs],
                                        512, eps_col, "x")
                nc.gpsimd.partition_broadcast(a_b[:, cs], arow[:])
                nc.gpsimd.partition_broadcast(b_b[:, cs], brow[:])

            x_sum(0)
            x_sqsum(0)
            x_sum(1)
            x_sum(2)
            x_rowstats(0)

            # ---- apply folded LN to hc (gates FF) on the DVE ----
            a_bT = a_b[0:P, 0:TOWN]
            b_bT = b_b[0:P, 0:TOWN]
            for i in range(ND):
                tm2 = xcp.tile([P, TOWN], F16, tag=f"htm{i % 2}", name=f"htm{i % 2}")
                nc.vector.tensor_mul(tm2[:], xcs[i][:, 0:TOWN], a_bT)
                nc.vector.tensor_add(hc[i][:], tm2[:], b_bT)

            # ---- PE: q/kv projections on raw x while the stats drain ----
            # (psum copies ride the scalar engine; DVE owns the fold path)
            for fb in range(ND):
                load_wq1(fb + 2)
                ps = s1ps.tile([P, TOWN], F32, tag="mm", name="mm")
                for d in range(ND):
                    nc.tensor.matmul(ps[:], wq1_tiles[fb][:, d, :], xcs[d][:, 0:TOWN],
                                     start=(d == 0), stop=(d == ND - 1))
                del wq1_tiles[fb]
                nc.scalar.copy(qT[fb][:], ps[:])
                if fb == 3:
                    x_sqsum(1)
                    x_rowstats(1)
                if fb == 5:
                    x_sqsum(2)
                    x_rowstats(2)
            wkv1 = wq1p.tile([P, ND, P], F16, tag="wq1", name="wq1")
            nc.sync.dma_start(wkv1[:], wkv1_d[:])
            kvs = s0.tile([P, TALL], F16, tag="kvs", name="kvs")
            for ch in range(2):
                cs = slice(TOWN + ch * 512, TOWN + (ch + 1) * 512)
                ps = s1ps.tile([P, 512], F32, tag="mm", name="mm")
                for d in range(ND):
                    nc.tensor.matmul(ps[:], wkv1[:, d, :], xcs[d][:, cs],
                                     start=(d == 0), stop=(d == ND - 1))
                nc.scalar.copy(kvs[:, ch * 512:(ch + 1) * 512], ps[:])

            # ---- S0 tail, emitted piecewise between FF chunks below ----
            def tail_kv():
                tqk = s0.tile([P, TALL], F16, tag="tqk", name="tqk")
                nc.scalar.activation(tqk[:], b_b[0:P, TOWN:TOT], AF.Copy,
                                     scale=ckv1_sb[:])
                tmk = s0.tile([P, TALL], F16, tag="tmkv", name="tmkv")
                nc.vector.tensor_mul(tmk[:], kvs[:], a_b[0:P, TOWN:TOT])
                nc.vector.tensor_add(kvs[:], tmk[:], tqk[:])
                nc.vector.tensor_copy(kb[0:DH, :], kvs[0:DH, :])
                nc.sync.dma_start(kb[DH:P, :], kvs[0:DH, :])
                for j in range(NJB):
                    tp = vtps.tile([P, DH], F16, tag="vt", name="vt")
                    nc.tensor.transpose(tp[:, :], kvs[DH:P, j * P:(j + 1) * P],
                                        ident[DH:P, DH:P])
                    nc.vector.tensor_copy(v_aug[j][:, 0:DH], tp[:, :])
                    nc.gpsimd.memset(v_aug[j][:, DH:DH + 1], 1.0)

            def tail_ctx():
                ps = stps.tile([1, 512], F32, tag="st", name="st")
                for d in range(ND):
                    nc.tensor.matmul(ps[:], ones_h[:], ctf[d][:],
                                     start=(d == 0), stop=(d == ND - 1))
                nc.vector.tensor_copy(csrow[:], ps[:])
                csq = [s0.tile([P, 512], F16, tag=f"sqx{d % 2}", name=f"sqx{d % 2}")
                       for d in range(ND)]
                for d in range(ND):
                    nc.vector.tensor_mul(csq[d][:], ctf[d][:], ctf[d][:])
                ps = stps.tile([1, 512], F32, tag="st", name="st")
                for d in range(ND):
                    nc.tensor.matmul(ps[:], ones_h[:], csq[d][:],
                                     start=(d == 0), stop=(d == ND - 1))
                nc.vector.tensor_copy(cqrow[:], ps[:])
                acrow, bcrow = _row_stats(nc, s0, csrow, cqrow, CTX, eps_col, "c")
                acb = s0.tile([P, CTX], F16, tag="acb", name="acb")
                bcb = s0.tile([P, CTX], F16, tag="bcb", name="bcb")
                nc.gpsimd.partition_broadcast(acb[:], acrow[:])
                nc.gpsimd.partition_broadcast(bcb[:], bcrow[:])
                tail_ctx.acb, tail_ctx.bcb = acb, bcb

            def tail_kv2():
                acb, bcb = tail_ctx.acb, tail_ctx.bcb
                wkv2 = s0.tile([P, ND, P], F16, tag="wkv2", name="wkv2")
                nc.sync.dma_start(wkv2[:], wkv2_d[:])
                ps_kv2 = s1ps.tile([P, CTX], F32, tag="mm", name="mm")
                for d in range(ND):
                    nc.tensor.matmul(ps_kv2[:], wkv2[:, d, :], ctf[d][:],
                                     start=(d == 0), stop=(d == ND - 1))
                kv2s = s0.tile([P, CTX], F16, tag="kv2s", name="kv2s")
                tkc = s0.tile([P, CTX], F16, tag="tkc", name="tkc")
                nc.scalar.activation(tkc[:], bcb[:], AF.Copy, scale=ckv2_sb[:])
                tmc = s0.tile([P, CTX], F16, tag="tmc", name="tmc")
                nc.vector.tensor_mul(tmc[:], ps_kv2[:], acb[:])
                nc.vector.tensor_add(kv2s[:], tmc[:], tkc[:])
                nc.scalar.copy(k2b[0:DH, :], kv2s[0:DH, :])
                nc.sync.dma_start(k2b[DH:P, :], kv2s[0:DH, :])
                for j in range(NJ2):
                    tp = vtps.tile([P, DH], F16, tag="vt", name="vt")
                    nc.tensor.transpose(tp[:, :], kv2s[DH:P, j * P:(j + 1) * P],
                                        ident[DH:P, DH:P])
                    nc.vector.tensor_copy(v2_aug[j][:, 0:DH], tp[:, :])
                    nc.gpsimd.memset(v2_aug[j][:, DH:DH + 1], 1.0)

            def tail_qfold(lo, hi):
                for fb in range(lo, hi):
                    tq = s0.tile([P, TOWN], F16, tag="tkc", name="tkc")
                    nc.scalar.activation(tq[:], b_bT, AF.Copy,
                                         scale=cq1_sb[:, fb:fb + 1])
                    tm = s0.tile([P, TOWN], F16, tag="tmc", name="tmc")
                    nc.vector.tensor_mul(tm[:], qT[fb][:], a_bT)
                    nc.vector.tensor_add(qT[fb][:], tm[:], tq[:])

            tail = [tail_kv, tail_ctx, tail_kv2,
                    lambda: tail_qfold(0, 4), lambda: tail_qfold(4, ND)]

            _xcp_cm.__exit__(None, None, None)  # frees x tiles for sgT/wfop

            # ---- S2a FF loop (silu-only on scalar engine) ----
            for gc in range(8):
                load_wff(gc + 2)
                w = wff_tiles[gc]
                for sub in range(4):
                    psa = ffa_ps.tile([P, TOWN], F32, tag="psa", name="psa")
                    for d in range(ND):
                        nc.tensor.matmul(psa[:], w[:, d, sub * P:(sub + 1) * P],
                                         hc[d][:],
                                         start=(d == 0), stop=(d == ND - 1))
                    psg = ffg_ps.tile([P, TOWN], F32, tag="psg", name="psg")
                    for d in range(ND):
                        nc.tensor.matmul(psg[:], w[:, d, 512 + sub * P:512 + (sub + 1) * P],
                                         hc[d][:],
                                         start=(d == 0), stop=(d == ND - 1))
                    g = gc * 4 + sub
                    sil = silp.tile([P, TOWN], F16, tag="sil", name="sil")
                    nc.scalar.activation(sil[:], psg[:], AF.Silu)
                    sgT[g] = sgp.tile([P, TOWN], F16, tag=f"sg{g}", name=f"sg{g}")
                    nc.vector.tensor_mul(sgT[g][:], sil[:], psa[:])
                del wff_tiles[gc]
                if gc - 1 < len(tail) and gc >= 1:
                    tail[gc - 1]()
                if gc == 5:
                    load_wfo(0)
                    load_wfo(1)

          _wffp_cm.__exit__(None, None, None)  # FF weights done

          with tc.tile_pool(name="xop", bufs=1) as xop, \
               tc.tile_pool(name="wyp", bufs=8) as wyp:
            xo = [xop.tile([P, TOWN], F32, tag=f"xo{i}", name=f"xo{i}")
                  for i in range(ND)]
            out2T = [xop.tile([P, TOWN], F16, tag=f"o2{i}", name=f"o2{i}") for i in range(ND)]
            yT = [xop.tile([P, TOWN], F16, tag=f"yT{i}", name=f"yT{i}") for i in range(ND)]
            yff = [xop.tile([P, TOWN], F16, tag=f"yf{i}", name=f"yf{i}") for i in range(ND)]
            wattn_tiles = {}

            def load_wy(fp):
                if fp < ND:
                    wa = wyp.tile([P, ND, P], F16, tag="wat", name="wat")
                    nc.sync.dma_start(wa[:], w4(wattn_d, fp))
                    wattn_tiles[fp] = wa

            # ====== S2b: self-attention (exp-only scalar engine) ============
            # Per key-block: sim -> exp -> 0/1 mask multiply, AV deferred by
            # TWO blocks so exp+mask hide behind sim+av+blob PE work.  Both
            # heads' AV + denominators land in one psum tile via the shared
            # MQA v.  The 32-matmul ff_out blob is spread through the loop.
            with tc.tile_pool(name="simp", bufs=2, space="PSUM") as sim_ps, \
                 tc.tile_pool(name="avpp", bufs=1, space="PSUM") as av_ps, \
                 tc.tile_pool(name="blobp", bufs=2, space="PSUM") as blobp, \
                 tc.tile_pool(name="probsp", bufs=4) as probsp, \
                 tc.tile_pool(name="nrm", bufs=1) as nrm:
                for i in range(ND):
                    nc.sync.dma_start(xo[i][:], xo_d[i * P:(i + 1) * P, :])
                load_wy(0)
                load_wy(1)
                for fb in range(ND):
                    avp = av_ps.tile([P, 2, TOWN], F32, tag="avp", name="avp")
                    blob = blobp.tile([P, TOWN], F32, tag="blob", name="blob")
                    pend = []     # (jb, probs, co, ncl) awaiting AV, depth 3
                    gseq = [0]    # next ff_out blob matmul

                    def blob_mms(k):
                        while gseq[0] < min(k, NG):
                            g = gseq[0]
                            nc.tensor.matmul(blob[:, :], wffo_tiles[fb][:, g, :],
                                             sgT[g][:], start=(g == 0),
                                             stop=(g == NG - 1))
                            gseq[0] += 1

                    def do_av(item):
                        jb, probs, co, ncl = item
                        st = (jb == 0)
                        sp = (jb == NJB - 1)
                        nc.tensor.matmul(avp[0:DH + 1, 0, co:TOWN],
                                         v_aug[jb][:, 0:DH + 1], probs[:, 0, 0:ncl],
                                         start=st, stop=sp)
                        nc.tensor.matmul(avp[0:DH + 1, 1, co:TOWN],
                                         v_aug[jb][:, 0:DH + 1], probs[:, 1, 0:ncl],
                                         start=st, stop=sp)

                    for jb in range(NJB):
                        co = (jb // 2) * P
                        ncl = TOWN - co
                        sim = sim_ps.tile([P, 2, TOWN], F32, tag="sim", name="sim")
                        nc.tensor.matmul(sim[:, 0, 0:ncl], kb[0:DH, jb * P:(jb + 1) * P],
                                         qT[fb][0:DH, co:TOWN], start=True, stop=True)
                        nc.tensor.matmul(sim[:, 1, 0:ncl], kb[DH:P, jb * P:(jb + 1) * P],
                                         qT[fb][DH:P, co:TOWN], start=True, stop=True)
                        probs = probsp.tile([P, 2, TOWN], F16, tag="probs", name="probs")
                        nc.scalar.activation(probs[:, :, 0:ncl], sim[:, :, 0:ncl], AF.Exp)
                        nc.vector.tensor_mul(probs[:, 0, 0:P], probs[:, 0, 0:P],
                                             mask_sb[:, jb, :])
                        nc.vector.tensor_mul(probs[:, 1, 0:P], probs[:, 1, 0:P],
                                             mask_sb[:, jb, :])
                        if len(pend) >= 3:
                            do_av(pend.pop(0))
                        pend.append((jb, probs, co, ncl))
                        blob_mms(3 * (jb + 1))
                    for item in pend:
                        do_av(item)
                    blob_mms(NG)  # tail covers this fb's norm chain
                    attn_norm(nc, nrm, avp, out2T[fb], "n1")
                    nc.vector.tensor_copy(yff[fb][:], blob[:, :])
                    del wffo_tiles[fb]
                    load_wfo(fb + 2)
                    if fb >= 2:
                        load_wy(fb)

            # ============ S3: y = x + attn_out + yff; stats; q2 =============
            with tc.tile_pool(name="s3", bufs=1) as s3, \
                 tc.tile_pool(name="sqyp", bufs=8) as sqyp, \
                 tc.tile_pool(name="wq2p", bufs=8) as wq2p, \
                 tc.tile_pool(name="s3ps", bufs=4, space="PSUM") as s3ps, \
                 tc.tile_pool(name="ysps", bufs=1, space="PSUM") as ysps:
                wq2_tiles = {}

                def load_wq2(fb):
                    if fb < ND:
                        t = wq2p.tile([P, ND, P], F16, tag="wq2", name="wq2")
                        nc.sync.dma_start(t[:], w4(wq2_d, fb))
                        wq2_tiles[fb] = t

                load_wq2(0)
                load_wq2(1)
                ysum = ysps.tile([1, TOWN], F32, tag="ysum", name="ysum")
                ysq = ysps.tile([1, TOWN], F32, tag="ysq", name="ysq")
                # pass 1: attn_out projections; y assembled by trailing DVE adds
                sqys = []
                for fp in range(ND):
                    psy = s3ps.tile([P, TOWN], F32, tag="psy", name="psy")
                    for d in range(ND):
                        nc.tensor.matmul(psy[:], wattn_tiles[fp][:, d, :], out2T[d][:],
                                         start=(d == 0), stop=(d == ND - 1))
                    del wattn_tiles[fp]
                    ty = s3.tile([P, TOWN], F32, tag=f"ty{fp % 2}", name=f"ty{fp % 2}")
                    nc.vector.tensor_add(ty[:], psy[:], xo[fp][:])
                    nc.vector.tensor_add(yT[fp][:], ty[:], yff[fp][:])
                    sqy = sqyp.tile([P, TOWN], F16, tag="sqy", name="sqy")
                    nc.scalar.activation(sqy[:], yT[fp][:], AF.Square)
                    sqys.append(sqy)
                    load_wq2(fp + 2)
                # pass 2: y stats (PE is 16 tiny matmuls; stats chain follows)
                for fp in range(ND):
                    nc.tensor.matmul(ysum[:], ones_h[:], yT[fp][:],
                                     start=(fp == 0), stop=(fp == ND - 1))
                    nc.tensor.matmul(ysq[:], ones_h[:], sqys[fp][:],
                                     start=(fp == 0), stop=(fp == ND - 1))
                a2h, b2h = _row_stats(nc, s3, ysum, ysq, TOWN, eps_col, "y")
                nc.gpsimd.partition_broadcast(a2b[:], a2h[:])
                nc.gpsimd.partition_broadcast(b2b[:], b2h[:])
                # pass 3: q2 projection (raw) + LN fold; the deep psum ring
                # keeps the PE clear of the trailing fold chain
                for fb in range(ND):
                    ps = s3ps.tile([P, TOWN], F32, tag="psy", name="psy")
                    for d in range(ND):
                        nc.tensor.matmul(ps[:], wq2_tiles[fb][:, d, :], yT[d][:],
                                         start=(d == 0), stop=(d == ND - 1))
                    del wq2_tiles[fb]
                    tq2 = s3.tile([P, TOWN], F16, tag=f"tq2{fb % 2}", name=f"tq2{fb % 2}")
                    nc.scalar.activation(tq2[:], b2b[:], AF.Copy,
                                         scale=c2_sb[:, fb:fb + 1])
                    tm2 = s3.tile([P, TOWN], F16, tag=f"tm2{fb % 2}", name=f"tm2{fb % 2}")
                    nc.vector.tensor_mul(tm2[:], ps[:], a2b[:])
                    nc.vector.tensor_add(q2T[fb][:], tm2[:], tq2[:])

        # ============ S4: cross-attention; S5: out = W_out^T @ out2b ========
        # S5 runs as two half-contractions: d=0..3 psums ride the scalar-bound
        # back half of S4 (stashed to SBUF), d=4..7 + the stash add at the end.
        with tc.tile_pool(name="wop", bufs=8) as wop:
            wo_tiles = {}

            def load_wo(fp):
                if fp < ND:
                    t = wop.tile([P, ND, P], F16, tag="wo", name="wo")
                    nc.sync.dma_start(t[:], w4(wout_d, fp))
                    wo_tiles[fp] = t

            load_wo(0)
            load_wo(1)
            with tc.tile_pool(name="sim2p", bufs=2, space="PSUM") as sim2_ps, \
                 tc.tile_pool(name="probs2p", bufs=3) as probs2p, \
                 tc.tile_pool(name="avp2p", bufs=2, space="PSUM") as av2_ps, \
                 tc.tile_pool(name="nrm2", bufs=1) as nrm2:
                for fb in range(ND):
                    avp = av2_ps.tile([P, 2, TOWN], F32, tag="avp2", name="avp2")
                    pend = []

                    def do_av2(item):
                        jb, probs = item
                        st = (jb == 0)
                        sp = (jb == NJ2 - 1)
                        nc.tensor.matmul(avp[0:DH + 1, 0, :], v2_aug[jb][:, 0:DH + 1],
                                         probs[:, 0, :], start=st, stop=sp)
                        nc.tensor.matmul(avp[0:DH + 1, 1, :], v2_aug[jb][:, 0:DH + 1],
                                         probs[:, 1, :], start=st, stop=sp)

                    for jb in range(NJ2):
                        sim = sim2_ps.tile([P, 2, TOWN], F32, tag="sim2", name="sim2")
                        nc.tensor.matmul(sim[:, 0, :], k2b[0:DH, jb * P:(jb + 1) * P],
                                         q2T[fb][0:DH, :], start=True, stop=True)
                        nc.tensor.matmul(sim[:, 1, :], k2b[DH:P, jb * P:(jb + 1) * P],
                                         q2T[fb][DH:P, :], start=True, stop=True)
                        probs = probs2p.tile([P, 2, TOWN], F16, tag="probs2", name="probs2")
                        nc.scalar.activation(probs[:], sim[:], AF.Exp)
                        if len(pend) >= 2:
                            do_av2(pend.pop(0))
                        pend.append((jb, probs))
                    for item in pend:
                        do_av2(item)
                    if fb >= 2:
                        load_wo(fb)
                    attn_norm(nc, nrm2, avp, out2b[fb], "n2")

            with tc.tile_pool(name="s5", bufs=2) as s5, \
                 tc.tile_pool(name="s5ps", bufs=2, space="PSUM") as s5ps:
                for fp in range(ND):
                    ps = s5ps.tile([P, TOWN], F32, tag="o", name="o")
                    for d in range(ND):
                        nc.tensor.matmul(ps[:], wo_tiles[fp][:, d, :], out2b[d][:],
                                         start=(d == 0), stop=(d == ND - 1))
                    del wo_tiles[fp]
                    osb = s5.tile([P, TOWN], F32, tag="osb", name="osb")
                    nc.scalar.copy(osb[:], ps[:])
                    nc.sync.dma_start(out_d[fp * P:(fp + 1) * P, :], osb[:])

    nc.compile()
    return nc


# --------------------------------------------------------------------------
# host side
# --------------------------------------------------------------------------

def _tile_w(w, pdim=P):
    """[D_in, D_out] -> [nfb, P, nd, P] with [fb, p, d, f] = w[d*P+p, fb*P+f]."""
    di, do = w.shape
    ndt, nfb = di // pdim, do // pdim
    return np.ascontiguousarray(
        w.reshape(ndt, pdim, nfb, pdim).transpose(2, 1, 0, 3))


def _prep_inputs(inputs):
    f32, f16 = np.float32, np.float16
    x = np.asarray(inputs["x"], dtype=f32)
    context = np.asarray(inputs["context"], dtype=f32)
    ptb_gamma = np.asarray(inputs["ptb_gamma"], dtype=f32)
    W_fused = np.asarray(inputs["W_fused"], dtype=f32)
    W_attn_out = np.asarray(inputs["W_attn_out"], dtype=f32)
    W_ff_out = np.asarray(inputs["W_ff_out"], dtype=f32)
    ca_gamma = np.asarray(inputs["ca_gamma"], dtype=f32)
    ca_ctx_gamma = np.asarray(inputs["ca_ctx_gamma"], dtype=f32)
    W_q = np.asarray(inputs["W_q"], dtype=f32)
    W_kv = np.asarray(inputs["W_kv"], dtype=f32)
    W_out = np.asarray(inputs["W_out"], dtype=f32)

    Wf = ptb_gamma[:, None] * W_fused
    wq1 = Wf[:, :D] * SCALE                      # [1024, 1024]
    wkv1 = Wf[:, D:D + 2 * DH]                   # [1024, 128]
    wff_a = Wf[:, D + 2 * DH:D + 2 * DH + 4096]  # [1024, 4096]
    wff_g = Wf[:, D + 2 * DH + 4096:]            # [1024, 4096]
    wff = np.empty((8, P, ND, 1024), dtype=f16)
    wff[..., 0:512] = wff_a.reshape(ND, P, 8, 512).transpose(2, 1, 0, 3)
    wff[..., 512:1024] = wff_g.reshape(ND, P, 8, 512).transpose(2, 1, 0, 3)

    wq2_eff = ca_gamma[:, None] * W_q * SCALE
    wkv2_eff = ca_ctx_gamma[:, None] * W_kv
    cq1 = wq1.sum(axis=0)                        # [1024]
    ckv1 = wkv1.sum(axis=0)                      # [128]
    c2 = wq2_eff.sum(axis=0)                     # [1024]
    ckv2 = wkv2_eff.sum(axis=0)                  # [128]

    wffo = np.ascontiguousarray(
        W_ff_out.reshape(NG, P, ND, P).transpose(2, 1, 0, 3)).astype(f16)

    shared = {
        "wq1": _tile_w(wq1).astype(f16),
        "wkv1": np.ascontiguousarray(
            wkv1.reshape(ND, P, 2 * DH).transpose(1, 0, 2)).astype(f16),
        "wff": wff,
        "wattn": _tile_w(W_attn_out).astype(f16),
        "wffo": wffo,
        "wq2": _tile_w(wq2_eff).astype(f16),
        "wkv2": np.ascontiguousarray(
            wkv2_eff.reshape(ND, P, 2 * DH).transpose(1, 0, 2)).astype(f16),
        "wout": _tile_w(W_out).astype(f16),
        "cq1": np.ascontiguousarray(cq1.reshape(ND, P).T).astype(f32),
        "ckv1": ckv1.reshape(P, 1).astype(f32),
        "c2": np.ascontiguousarray(c2.reshape(ND, P).T).astype(f32),
        "ckv2": ckv2.reshape(P, 1).astype(f32),
    }

    jj = np.arange(P)
    tri = (jj[:, None] <= jj[None, :]).astype(f16)   # probs[key, query] keep
    in_maps, own_idx_all = [], []
    for b in range(4):
        xb = x[b]
        cb = context[b]
        for s in range(2):
            own_idx = np.concatenate([np.arange(g * P, (g + 1) * P)
                                      for g in (s, s + 2, s + 4, s + 6)])
            own_idx_all.append(own_idx)
            xown = xb[own_idx]                       # [512, 1024]
            xt = np.concatenate([xown, xb], axis=0)  # [1536, 1024] own-first
            mask = np.zeros((P, NJB, P), dtype=f16)
            for jb in range(NJB):
                gb = s + 2 * (jb // 2)   # own block sitting at the first slot
                if gb > jb:
                    mask[:, jb, :] = 1.0
                elif gb == jb:
                    mask[:, jb, :] = tri
                # gb < jb: stays 0.0 (fully masked)
            m = dict(shared)
            m.update({
                "xc": np.ascontiguousarray(xt.T).astype(f16),
                "xo": np.ascontiguousarray(xown.T),
                "ctf": np.ascontiguousarray(cb.T).astype(f16),
                "mask": mask,
            })
            in_maps.append(m)
    return in_maps, own_idx_all


def _install_trace_hook():
    import types
    if "antenv.axon_hooks" not in sys.modules:
        mod = types.ModuleType("antenv.axon_hooks")
        _HOOK = [None]
        mod.get_axon_ntff_profile_hook = lambda: _HOOK[0]
        mod.set_axon_ntff_profile_hook = lambda h: _HOOK.__setitem__(0, h)
        sys.modules["antenv.axon_hooks"] = mod
        import antenv
        antenv.axon_hooks = mod
    mod = sys.modules["antenv.axon_hooks"]
    if mod.get_axon_ntff_profile_hook() is None:
        from trn_agent_boot.trn_boot import _ntff_profile_via_ctypes
        mod.set_axon_ntff_profile_hook(
            _ntff_profile_via_ctypes("/opt/axon/libaxon_pjrt.so"))


def kernel(**inputs):
    global LAST_EXEC_NS
    if "nc" not in _CACHE:
        _CACHE["nc"] = build_module()
    nc = _CACHE["nc"]
    in_maps, own_idx_all = _prep_inputs(inputs)

    kwargs = {}
    if TRACE:
        try:
            _install_trace_hook()
            kwargs["trace"] = True
        except Exception as e:
            print(f"trace setup failed: {e}", file=sys.stderr)

    res = run_bass_kernel_spmd(nc, in_maps, core_ids=list(range(NCORES)), **kwargs)
    LAST_EXEC_NS = res.exec_time_ns
    if TRACE and res.instructions_and_trace:
        _CACHE["trace_path"] = res.instructions_and_trace[1]

    out = np.empty((4, TALL, D), dtype=np.float32)
    for core in range(NCORES):
        outT = res.results[core]["outT"]
        out[core // 2, own_idx_all[core], :] = outT.T
    return out
